# revision 20
# baseline (speedup 1.0000x reference)
# GCN layer kernel for Trainium2: out[b] = relu((a[b] @ x[b]) @ W) * mask[b]
#
# Sharding: data-parallel over the batch (graph) dim. B=8 graphs, 8 cores,
# one graph per core; W replicated. Inputs are the FULL tensors; shards are
# sliced host-side and the per-core outputs stacked back together.
#
# Host-side data prep (part of the shard step): a[b] is transposed to
# aT[m, n] and all matmul operands are cast to bf16. TensorE contracts over
# the partition (row) index of both operands, so a@x needs a's column index
# (m) on partitions -- feeding aT directly removes the 256 on-chip PE
# transposes (and their PSUM->SBUF copybacks) that dominated the fp32
# version's TensorE time. bf16 also halves HBM traffic for a (the dominant
# tensor), enables FWL weight loads, and needs no walrus f32r rounding
# copies; accuracy lands ~3e-3 rel vs the 2e-2 gate (fp32 PSUM accumulate).
#
# Per-core dataflow (aT: [2048,2048], x: [2048,512], W: [512,512]):
#   - mm0: t^T[f, nc] = sum_m x[m, f] * aT[m, nc]: lhsT = x tile [128m,128f]
#     (stationary), rhs = aT tile [128m, 512n] (moving), PSUM [128f, 512n],
#     accumulated over 16 m-tiles into one of 4 f-banks. n is processed in
#     4 chunks of 512 (PSUM bank = 512 fp32).
#   - tt copyback: PSUM f32 -> SBUF bf16 on DVE (mm2's lhsT).
#   - mm2: out[n, d] = sum_f t^T[f, n] * W[f, d]: lhsT = tt tile, rhs = W,
#     PSUM [128n, 512d] accumulated over the 4 f-tiles -> direct [n,d] store.
#   - mask[n] = any(x[n,:] != 0), applied fused into the ReLU via scale=.
#
# Schedule notes (from NTFF traces; steady-state MM issue gap measures
# 216 ns = the N=512 warm-clock limit, so all tuning is in the edges):
#   - Loads ride three DMA queues: a-chunks on Sync (4 x 512KB per chunk),
#     x ROW-GROUP chunks on Scalar (row-major rows -> 1KB descriptors;
#     column-chunks made 256B descriptors and ran ~6x slower), W on GpSimd.
#     Every descriptor here is >=1KB contiguous (line-rate needs >=512B).
#   - Chunk 0's mm0 runs mi-outer so each x row-group + a quarter-chunk
#     unlocks the next 16 matmuls: real MMs from ~8.7us keep the PE HAM
#     activity window dense (3 warm-up MMs on a memset tile bridge the
#     first DMA wait), so the clock gate opens ~11us and never re-drops.
#     The last m-tile is staggered per fi with its PSUM->SBUF copyback so
#     mm2 groups pipeline in with no PE stall. Chunks 1-3 run fi-outer
#     (x is resident) with mm2 of fi-1 between mm0 groups, same effect.
#   - mask reductions are gated per row-group (|x| on ACT for even
#     row-tiles, (x!=0)-count on DVE for odd) and finish by ~15us, far
#     ahead of the first ReLU.
#   - Stores for chunks 0-2 ride GpSimd (idle queue), but the LAST chunk's
#     stores go on Sync/Scalar (HWDGE): the SWDGE completion receipt is
#     ~5us and sat on the critical path at the kernel's end. Last-chunk
#     ReLUs alternate ACT/DVE so the two drain chains run in parallel.

import numpy as np

B, N, F, D = 8, 2048, 512, 512
P = 128
NT = N // P        # 16 m-tiles (and n row-tiles; a is square)
FT = F // P        # 4 f-tiles
NCHUNK = 512       # n chunk width (one PSUM bank of fp32)
NJ = N // NCHUNK   # 4
NSUB = NCHUNK // P # 4

_CACHE = {}


def _build_nc():
    from contextlib import ExitStack

    from concourse import bacc, mybir, tile

    f32 = mybir.dt.float32
    bf16 = mybir.dt.bfloat16
    AF = mybir.ActivationFunctionType
    ALU = mybir.AluOpType

    nc = bacc.Bacc(None)
    at_d = nc.dram_tensor("at", [N, N], bf16, kind="ExternalInput")  # a^T [m,n]
    x_d = nc.dram_tensor("x", [N, F], bf16, kind="ExternalInput")
    w_d = nc.dram_tensor("kernel", [F, D], bf16, kind="ExternalInput")
    o_d = nc.dram_tensor("out", [N, D], bf16, kind="ExternalOutput")

    with tile.TileContext(nc) as tc, ExitStack() as ctx:
        const = ctx.enter_context(tc.tile_pool(name="const", bufs=1))
        xp = ctx.enter_context(tc.tile_pool(name="xp", bufs=1))
        wp = ctx.enter_context(tc.tile_pool(name="wp", bufs=1))
        a_pool = ctx.enter_context(tc.tile_pool(name="a_pool", bufs=2))
        ttp = ctx.enter_context(tc.tile_pool(name="ttp", bufs=2))
        outp = ctx.enter_context(tc.tile_pool(name="outp", bufs=4))
        scr = ctx.enter_context(tc.tile_pool(name="scr", bufs=2))
        ps_t = ctx.enter_context(tc.tile_pool(name="ps_t", bufs=4, space="PSUM"))
        ps_o = ctx.enter_context(tc.tile_pool(name="ps_o", bufs=4, space="PSUM"))

        # Warm-up operand: junk bf16 tile (values irrelevant, PSUM discarded)
        wb = const.tile([P, P * 2], bf16)
        nc.vector.memset(wb[:], 1.0)

        def warm_mm():
            # 256-wide: half the cold-clock cycles per unit of HAM activity,
            # so the ~3.4us busy window that opens the clock gate is spent
            # before the first real matmul instead of during it
            pw = ps_o.tile([P, D], f32, tag="pso", name="pw")
            nc.tensor.matmul(
                pw[:, : P * 2], lhsT=wb[:, :P], rhs=wb[:],
                start=True, stop=True,
            )

        for _ in range(13):
            warm_mm()

        # Single-queue prologue on Sync, interleaved in consumption order
        # (x row-group g with a-chunk-0 quarter g, then W): one FIFO stream
        # at full HBM rate instead of three queues diluting each other.
        x_sb = xp.tile([P, NT, F], bf16)

        def load_x_group(g):
            nc.sync.dma_start(
                x_sb[:, g * 4 : (g + 1) * 4, :],
                x_d[g * 4 * P : (g + 1) * 4 * P, :].rearrange(
                    "(o p) f -> p o f", p=P
                ),
            )

        def load_a_quarter(at, nj, g):
            nc.sync.dma_start(
                at[:, g * 4 : (g + 1) * 4, :],
                at_d[
                    g * 4 * P : (g + 1) * 4 * P,
                    nj * NCHUNK : (nj + 1) * NCHUNK,
                ].rearrange("(o p) n -> p o n", p=P),
            )

        # First x/a tiles on PARALLEL queues (x on Scalar, a on Sync), with
        # the first two m-tiles as single-mi 128KB slices: the first tile's
        # DMA completion (data + ~2us receipt) gates the first real matmul,
        # and the two receipt pipelines overlap.
        at_cur = a_pool.tile([P, NT, NCHUNK], bf16, tag="at", name="at0")
        for lo, hi in ((0, 1), (1, 2), (2, 4)):
            nc.scalar.dma_start(
                x_sb[:, lo:hi, :],
                x_d[lo * P : hi * P, :].rearrange("(o p) f -> p o f", p=P),
            )
            nc.sync.dma_start(
                at_cur[:, lo:hi, :],
                at_d[lo * P : hi * P, 0:NCHUNK].rearrange(
                    "(o p) n -> p o n", p=P
                ),
            )
        for g in range(1, 4):
            load_x_group(g)
            load_a_quarter(at_cur, 0, g)
        w_sb = wp.tile([P, FT, D], bf16)
        nc.sync.dma_start(w_sb[:], w_d[:].rearrange("(o p) d -> p o d", p=P))

        # mask accumulators: |x| sums (even row-tiles, ACT) and nonzero
        # counts (odd row-tiles, DVE); is_gt merges both into mask_sb.
        sumabs = const.tile([P, NT], f32)
        mask_sb = const.tile([P, NT], f32)

        def mask_even(ni):
            abs_scr = scr.tile([P, F], bf16, tag="abs_scr")
            nc.scalar.activation(
                abs_scr[:], x_sb[:, ni], AF.Abs, accum_out=sumabs[:, ni : ni + 1]
            )

        def mask_odd(ni):
            ne_scr = scr.tile([P, F], bf16, tag="ne_scr")
            nc.vector.tensor_scalar(
                ne_scr[:], x_sb[:, ni], 0.0, None, ALU.not_equal, ALU.add,
                accum_out=sumabs[:, ni : ni + 1],
            )

        # n-chunk widths: the last 512 columns run as two 256-wide chunks so
        # the final relu/store drain chain is half-length (it sits entirely
        # on the critical path after the last matmul).
        WIDTHS = [512, 512, 512, 256, 256]
        OFFS = [0, 512, 1024, 1536, 1792]
        NCH = len(WIDTHS)

        def load_chunk(j):
            at = a_pool.tile([P, NT, NCHUNK], bf16, tag="at", name=f"at{j}")
            wdt, off = WIDTHS[j], OFFS[j]
            for g in range(4):
                nc.sync.dma_start(
                    at[:, g * 4 : (g + 1) * 4, :wdt],
                    at_d[g * 4 * P : (g + 1) * 4 * P, off : off + wdt].rearrange(
                        "(o p) n -> p o n", p=P
                    ),
                )
            return at

        for nj in range(NCH):
            at_next = load_chunk(nj + 1) if nj + 1 < NCH else None
            wdt, off = WIDTHS[nj], OFFS[nj]
            nsub = wdt // P
            tail2 = nj >= NCH - 2  # last two chunks: HWDGE stores, split relu

            tt = ttp.tile([P, FT, NCHUNK], bf16, tag="tt")
            pt = [
                ps_t.tile([P, NCHUNK], f32, tag="pst", name=f"pt{nj}_{fi}")
                for fi in range(FT)
            ]
            po = [
                ps_o.tile([P, D], f32, tag="pso", name=f"po{nj}_{ns}")
                for ns in range(nsub)
            ]

            def mm2_group(fi):
                for ns in range(nsub):
                    nc.tensor.matmul(
                        po[ns][:],
                        lhsT=tt[:, fi, ns * P : (ns + 1) * P],
                        rhs=w_sb[:, fi, :],
                        start=(fi == 0),
                        stop=(fi == FT - 1),
                    )

            if nj == 0:
                # mi-outer: x row-group g + a quarter-chunk g unlock the
                # next 16 MMs, so the PE starts as soon as the first 512KB
                # of each stream lands. Mask reductions ride per-group.
                for mi in range(NT - 1):
                    for fi in range(FT):
                        nc.tensor.matmul(
                            pt[fi][:],
                            lhsT=x_sb[:, mi, fi * P : (fi + 1) * P],
                            rhs=at_cur[:, mi, :],
                            start=(mi == 0),
                            stop=False,
                        )
                    if mi in (1, 3, 5):
                        # in-stream warm-ups: when the next mi's DMA isn't
                        # in yet, the PE chews these instead of idling, so
                        # the HAM activity window stays dense while the
                        # prologue streams in
                        warm_mm()
                    if mi % 4 == 3:
                        g = mi // 4
                        for ni in (4 * g, 4 * g + 2):
                            mask_even(ni)
                        for ni in (4 * g + 1, 4 * g + 3):
                            mask_odd(ni)
                # last m-tile staggered per fi: copybacks pipeline with the
                # mm2 groups two fi behind, so the PE never waits on DVE
                for fi in range(FT):
                    nc.tensor.matmul(
                        pt[fi][:],
                        lhsT=x_sb[:, NT - 1, fi * P : (fi + 1) * P],
                        rhs=at_cur[:, NT - 1, :],
                        start=False,
                        stop=True,
                    )
                    nc.vector.tensor_copy(tt[:, fi, :], pt[fi][:])
                    if fi == 0:
                        for ni in (12, 14):
                            mask_even(ni)
                        for ni in (13, 15):
                            mask_odd(ni)
                    if fi >= 2:
                        mm2_group(fi - 2)
                # mask merges (DVE): odd columns from DVE counts, even from
                # the ACT |x| chain
                nc.vector.tensor_scalar(
                    mask_sb[:, 1:NT:2], sumabs[:, 1:NT:2], 0.0, None, ALU.is_gt
                )
                nc.vector.tensor_scalar(
                    mask_sb[:, 0:NT:2], sumabs[:, 0:NT:2], 0.0, None, ALU.is_gt
                )
                mm2_group(FT - 2)
                mm2_group(FT - 1)
            else:
                def mm0_group(fi):
                    for mi in range(NT):
                        nc.tensor.matmul(
                            pt[fi][:, :wdt],
                            lhsT=x_sb[:, mi, fi * P : (fi + 1) * P],
                            rhs=at_cur[:, mi, :wdt],
                            start=(mi == 0),
                            stop=(mi == NT - 1),
                        )

                for fi in range(FT):
                    mm0_group(fi)
                    nc.vector.tensor_copy(tt[:, fi, :wdt], pt[fi][:, :wdt])
                    if fi >= 1:
                        mm2_group(fi - 1)
                mm2_group(FT - 1)

            for ns in range(nsub):
                ni = off // P + ns
                ob = outp.tile([P, D], bf16, tag="ob")
                if tail2 and ns % 2 == 1:
                    # parallel drain: DVE relu+mask, store via Scalar HWDGE
                    nc.vector.tensor_scalar(
                        ob[:], po[ns][:], 0.0, mask_sb[:, ni : ni + 1],
                        ALU.max, ALU.mult,
                    )
                    nc.scalar.dma_start(o_d[ni * P : (ni + 1) * P, :], ob[:])
                else:
                    nc.scalar.activation(
                        ob[:], po[ns][:], AF.Relu, scale=mask_sb[:, ni : ni + 1]
                    )
                    if tail2:
                        nc.sync.dma_start(o_d[ni * P : (ni + 1) * P, :], ob[:])
                    else:
                        nc.gpsimd.dma_start(o_d[ni * P : (ni + 1) * P, :], ob[:])

            at_cur = at_next

    nc.compile()
    return nc


def get_nc():
    if "nc" not in _CACHE:
        _CACHE["nc"] = _build_nc()
    return _CACHE["nc"]


def make_in_maps(x, a, w):
    """Host-side shard + layout prep: per-core aT/x/W in bf16."""
    import ml_dtypes

    bf = ml_dtypes.bfloat16
    w_bf = np.ascontiguousarray(w.astype(bf))
    in_maps = []
    for b in range(B):
        at_bf = np.ascontiguousarray(a[b].T.astype(bf))
        x_bf = np.ascontiguousarray(x[b].astype(bf))
        in_maps.append({"at": at_bf, "x": x_bf, "kernel": w_bf})
    return in_maps


def kernel(**inputs) -> np.ndarray:
    from concourse.bass_utils import run_bass_kernel_spmd

    x = np.asarray(inputs["x"], dtype=np.float32)
    a = np.asarray(inputs["a"], dtype=np.float32)
    w = np.asarray(inputs["kernel"], dtype=np.float32)
    assert x.shape == (B, N, F) and a.shape == (B, N, N) and w.shape == (F, D)

    nc = get_nc()
    in_maps = make_in_maps(x, a, w)
    res = run_bass_kernel_spmd(nc, in_maps, core_ids=list(range(B)))
    return np.stack(
        [np.asarray(res.results[b]["out"]).astype(np.float32) for b in range(B)],
        axis=0,
    )


# revision 21
# speedup vs baseline: 1.0046x; 1.0046x over previous
# GCN layer kernel for Trainium2: out[b] = relu((a[b] @ x[b]) @ W) * mask[b]
#
# Sharding: data-parallel over the batch (graph) dim. B=8 graphs, 8 cores,
# one graph per core; W replicated. Inputs are the FULL tensors; shards are
# sliced host-side and the per-core outputs stacked back together.
#
# Host-side data prep (part of the shard step): a[b] is transposed to
# aT[m, n] and all matmul operands are cast to bf16. TensorE contracts over
# the partition (row) index of both operands, so a@x needs a's column index
# (m) on partitions -- feeding aT directly removes the 256 on-chip PE
# transposes (and their PSUM->SBUF copybacks) that dominated the fp32
# version's TensorE time. bf16 also halves HBM traffic for a (the dominant
# tensor), enables FWL weight loads, and needs no walrus f32r rounding
# copies; accuracy lands ~3e-3 rel vs the 2e-2 gate (fp32 PSUM accumulate).
#
# Per-core dataflow (aT: [2048,2048], x: [2048,512], W: [512,512]):
#   - mm0: t^T[f, nc] = sum_m x[m, f] * aT[m, nc]: lhsT = x tile [128m,128f]
#     (stationary), rhs = aT tile [128m, 512n] (moving), PSUM [128f, 512n],
#     accumulated over 16 m-tiles into one of 4 f-banks. n is processed in
#     4 chunks of 512 (PSUM bank = 512 fp32).
#   - tt copyback: PSUM f32 -> SBUF bf16 on DVE (mm2's lhsT).
#   - mm2: out[n, d] = sum_f t^T[f, n] * W[f, d]: lhsT = tt tile, rhs = W,
#     PSUM [128n, 512d] accumulated over the 4 f-tiles -> direct [n,d] store.
#   - mask[n] = any(x[n,:] != 0), applied fused into the ReLU via scale=.
#
# Schedule notes (from NTFF traces; steady-state MM issue gap measures
# 216 ns = the N=512 warm-clock limit, so all tuning is in the edges):
#   - Loads ride three DMA queues: a-chunks on Sync (4 x 512KB per chunk),
#     x ROW-GROUP chunks on Scalar (row-major rows -> 1KB descriptors;
#     column-chunks made 256B descriptors and ran ~6x slower), W on GpSimd.
#     Every descriptor here is >=1KB contiguous (line-rate needs >=512B).
#   - Chunk 0's mm0 runs mi-outer so each x row-group + a quarter-chunk
#     unlocks the next 16 matmuls: real MMs from ~8.7us keep the PE HAM
#     activity window dense (3 warm-up MMs on a memset tile bridge the
#     first DMA wait), so the clock gate opens ~11us and never re-drops.
#     The last m-tile is staggered per fi with its PSUM->SBUF copyback so
#     mm2 groups pipeline in with no PE stall. Chunks 1-3 run fi-outer
#     (x is resident) with mm2 of fi-1 between mm0 groups, same effect.
#   - mask reductions are gated per row-group (|x| on ACT for even
#     row-tiles, (x!=0)-count on DVE for odd) and finish by ~15us, far
#     ahead of the first ReLU.
#   - Stores for chunks 0-2 ride GpSimd (idle queue), but the LAST chunk's
#     stores go on Sync/Scalar (HWDGE): the SWDGE completion receipt is
#     ~5us and sat on the critical path at the kernel's end. Last-chunk
#     ReLUs alternate ACT/DVE so the two drain chains run in parallel.

import numpy as np

B, N, F, D = 8, 2048, 512, 512
P = 128
NT = N // P        # 16 m-tiles (and n row-tiles; a is square)
FT = F // P        # 4 f-tiles
NCHUNK = 512       # n chunk width (one PSUM bank of fp32)
NJ = N // NCHUNK   # 4
NSUB = NCHUNK // P # 4

_CACHE = {}


def _build_nc():
    from contextlib import ExitStack

    from concourse import bacc, mybir, tile

    f32 = mybir.dt.float32
    bf16 = mybir.dt.bfloat16
    AF = mybir.ActivationFunctionType
    ALU = mybir.AluOpType

    nc = bacc.Bacc(None)
    at_d = nc.dram_tensor("at", [N, N], bf16, kind="ExternalInput")  # a^T [m,n]
    x_d = nc.dram_tensor("x", [N, F], bf16, kind="ExternalInput")
    w_d = nc.dram_tensor("kernel", [F, D], bf16, kind="ExternalInput")
    o_d = nc.dram_tensor("out", [N, D], bf16, kind="ExternalOutput")

    with tile.TileContext(nc) as tc, ExitStack() as ctx:
        const = ctx.enter_context(tc.tile_pool(name="const", bufs=1))
        xp = ctx.enter_context(tc.tile_pool(name="xp", bufs=1))
        wp = ctx.enter_context(tc.tile_pool(name="wp", bufs=1))
        a_pool = ctx.enter_context(tc.tile_pool(name="a_pool", bufs=2))
        ttp = ctx.enter_context(tc.tile_pool(name="ttp", bufs=2))
        outp = ctx.enter_context(tc.tile_pool(name="outp", bufs=4))
        scr = ctx.enter_context(tc.tile_pool(name="scr", bufs=2))
        ps_t = ctx.enter_context(tc.tile_pool(name="ps_t", bufs=4, space="PSUM"))
        ps_o = ctx.enter_context(tc.tile_pool(name="ps_o", bufs=4, space="PSUM"))

        # Warm-up operand: junk bf16 tile (values irrelevant, PSUM discarded)
        wb = const.tile([P, NCHUNK], bf16)
        nc.vector.memset(wb[:], 1.0)

        def warm_mm():
            # 256-wide: half the cold-clock cycles per unit of HAM activity,
            # so the ~3.4us busy window that opens the clock gate is spent
            # before the first real matmul instead of during it
            pw = ps_o.tile([P, D], f32, tag="pso", name="pw")
            nc.tensor.matmul(
                pw[:, : P * 2], lhsT=wb[:, :P], rhs=wb[:, : P * 2],
                start=True, stop=True,
            )

        for _ in range(14):
            warm_mm()

        # Single-queue prologue on Sync, interleaved in consumption order
        # (x row-group g with a-chunk-0 quarter g, then W): one FIFO stream
        # at full HBM rate instead of three queues diluting each other.
        x_sb = xp.tile([P, NT, F], bf16)

        def load_x_group(g):
            nc.sync.dma_start(
                x_sb[:, g * 4 : (g + 1) * 4, :],
                x_d[g * 4 * P : (g + 1) * 4 * P, :].rearrange(
                    "(o p) f -> p o f", p=P
                ),
            )

        def load_a_quarter(at, nj, g):
            nc.sync.dma_start(
                at[:, g * 4 : (g + 1) * 4, :],
                at_d[
                    g * 4 * P : (g + 1) * 4 * P,
                    nj * NCHUNK : (nj + 1) * NCHUNK,
                ].rearrange("(o p) n -> p o n", p=P),
            )

        # First x/a pair split into 256KB halves on PARALLEL queues (x on
        # Scalar, a on Sync): the first tile's DMA completion (data + ~2us
        # receipt) gates the first real matmul, and the two receipt
        # pipelines overlap.
        at_cur = a_pool.tile([P, NT, NCHUNK], bf16, tag="at", name="at0")
        for h in range(2):
            nc.scalar.dma_start(
                x_sb[:, h * 2 : (h + 1) * 2, :],
                x_d[h * 2 * P : (h + 1) * 2 * P, :].rearrange(
                    "(o p) f -> p o f", p=P
                ),
            )
            nc.sync.dma_start(
                at_cur[:, h * 2 : (h + 1) * 2, :],
                at_d[h * 2 * P : (h + 1) * 2 * P, 0:NCHUNK].rearrange(
                    "(o p) n -> p o n", p=P
                ),
            )
        for g in range(1, 4):
            load_x_group(g)
            load_a_quarter(at_cur, 0, g)
        w_sb = wp.tile([P, FT, D], bf16)
        nc.sync.dma_start(w_sb[:], w_d[:].rearrange("(o p) d -> p o d", p=P))

        # mask accumulators: |x| sums (even row-tiles, ACT) and nonzero
        # counts (odd row-tiles, DVE); is_gt merges both into mask_sb.
        sumabs = const.tile([P, NT], f32)
        mask_sb = const.tile([P, NT], f32)

        def mask_even(ni):
            abs_scr = scr.tile([P, F], bf16, tag="abs_scr")
            nc.scalar.activation(
                abs_scr[:], x_sb[:, ni], AF.Abs, accum_out=sumabs[:, ni : ni + 1]
            )

        def mask_odd(ni):
            ne_scr = scr.tile([P, F], bf16, tag="ne_scr")
            nc.vector.tensor_scalar(
                ne_scr[:], x_sb[:, ni], 0.0, None, ALU.not_equal, ALU.add,
                accum_out=sumabs[:, ni : ni + 1],
            )

        # n-chunk widths: the last 512 columns run as two 256-wide chunks so
        # the final relu/store drain chain is half-length (it sits entirely
        # on the critical path after the last matmul).
        WIDTHS = [512, 512, 512, 256, 256]
        OFFS = [0, 512, 1024, 1536, 1792]
        NCH = len(WIDTHS)

        def load_chunk(j):
            at = a_pool.tile([P, NT, NCHUNK], bf16, tag="at", name=f"at{j}")
            wdt, off = WIDTHS[j], OFFS[j]
            for g in range(4):
                nc.sync.dma_start(
                    at[:, g * 4 : (g + 1) * 4, :wdt],
                    at_d[g * 4 * P : (g + 1) * 4 * P, off : off + wdt].rearrange(
                        "(o p) n -> p o n", p=P
                    ),
                )
            return at

        for nj in range(NCH):
            at_next = load_chunk(nj + 1) if nj + 1 < NCH else None
            wdt, off = WIDTHS[nj], OFFS[nj]
            nsub = wdt // P
            tail2 = nj >= NCH - 2  # last two chunks: HWDGE stores, split relu

            tt = ttp.tile([P, FT, NCHUNK], bf16, tag="tt")
            pt = [
                ps_t.tile([P, NCHUNK], f32, tag="pst", name=f"pt{nj}_{fi}")
                for fi in range(FT)
            ]
            po = [
                ps_o.tile([P, D], f32, tag="pso", name=f"po{nj}_{ns}")
                for ns in range(nsub)
            ]

            def mm2_group(fi):
                for ns in range(nsub):
                    nc.tensor.matmul(
                        po[ns][:],
                        lhsT=tt[:, fi, ns * P : (ns + 1) * P],
                        rhs=w_sb[:, fi, :],
                        start=(fi == 0),
                        stop=(fi == FT - 1),
                    )

            if nj == 0:
                # mi-outer: x row-group g + a quarter-chunk g unlock the
                # next 16 MMs, so the PE starts as soon as the first 512KB
                # of each stream lands. Mask reductions ride per-group.
                for mi in range(NT - 1):
                    for fi in range(FT):
                        nc.tensor.matmul(
                            pt[fi][:],
                            lhsT=x_sb[:, mi, fi * P : (fi + 1) * P],
                            rhs=at_cur[:, mi, :],
                            start=(mi == 0),
                            stop=False,
                        )
                    if mi in (1, 3, 5):
                        # in-stream warm-ups: when the next mi's DMA isn't
                        # in yet, the PE chews these instead of idling, so
                        # the HAM activity window stays dense while the
                        # prologue streams in
                        warm_mm()
                    if mi % 4 == 3:
                        g = mi // 4
                        for ni in (4 * g, 4 * g + 2):
                            mask_even(ni)
                        for ni in (4 * g + 1, 4 * g + 3):
                            mask_odd(ni)
                # last m-tile staggered per fi: copybacks pipeline with the
                # mm2 groups two fi behind, so the PE never waits on DVE
                for fi in range(FT):
                    nc.tensor.matmul(
                        pt[fi][:],
                        lhsT=x_sb[:, NT - 1, fi * P : (fi + 1) * P],
                        rhs=at_cur[:, NT - 1, :],
                        start=False,
                        stop=True,
                    )
                    nc.vector.tensor_copy(tt[:, fi, :], pt[fi][:])
                    if fi == 0:
                        for ni in (12, 14):
                            mask_even(ni)
                        for ni in (13, 15):
                            mask_odd(ni)
                    if fi >= 2:
                        mm2_group(fi - 2)
                # mask merges (DVE): odd columns from DVE counts, even from
                # the ACT |x| chain
                nc.vector.tensor_scalar(
                    mask_sb[:, 1:NT:2], sumabs[:, 1:NT:2], 0.0, None, ALU.is_gt
                )
                nc.vector.tensor_scalar(
                    mask_sb[:, 0:NT:2], sumabs[:, 0:NT:2], 0.0, None, ALU.is_gt
                )
                mm2_group(FT - 2)
                mm2_group(FT - 1)
            else:
                def mm0_group(fi):
                    for mi in range(NT):
                        nc.tensor.matmul(
                            pt[fi][:, :wdt],
                            lhsT=x_sb[:, mi, fi * P : (fi + 1) * P],
                            rhs=at_cur[:, mi, :wdt],
                            start=(mi == 0),
                            stop=(mi == NT - 1),
                        )

                for fi in range(FT):
                    mm0_group(fi)
                    nc.vector.tensor_copy(tt[:, fi, :wdt], pt[fi][:, :wdt])
                    if fi >= 1:
                        mm2_group(fi - 1)
                mm2_group(FT - 1)

            for ns in range(nsub):
                ni = off // P + ns
                ob = outp.tile([P, D], bf16, tag="ob")
                if tail2 and ns % 2 == 1:
                    # parallel drain: DVE relu+mask, store via Scalar HWDGE
                    nc.vector.tensor_scalar(
                        ob[:], po[ns][:], 0.0, mask_sb[:, ni : ni + 1],
                        ALU.max, ALU.mult,
                    )
                    nc.scalar.dma_start(o_d[ni * P : (ni + 1) * P, :], ob[:])
                else:
                    nc.scalar.activation(
                        ob[:], po[ns][:], AF.Relu, scale=mask_sb[:, ni : ni + 1]
                    )
                    if tail2:
                        nc.sync.dma_start(o_d[ni * P : (ni + 1) * P, :], ob[:])
                    else:
                        nc.gpsimd.dma_start(o_d[ni * P : (ni + 1) * P, :], ob[:])

            at_cur = at_next

    nc.compile()
    return nc


def get_nc():
    if "nc" not in _CACHE:
        _CACHE["nc"] = _build_nc()
    return _CACHE["nc"]


def make_in_maps(x, a, w):
    """Host-side shard + layout prep: per-core aT/x/W in bf16."""
    import ml_dtypes

    bf = ml_dtypes.bfloat16
    w_bf = np.ascontiguousarray(w.astype(bf))
    in_maps = []
    for b in range(B):
        at_bf = np.ascontiguousarray(a[b].T.astype(bf))
        x_bf = np.ascontiguousarray(x[b].astype(bf))
        in_maps.append({"at": at_bf, "x": x_bf, "kernel": w_bf})
    return in_maps


def kernel(**inputs) -> np.ndarray:
    from concourse.bass_utils import run_bass_kernel_spmd

    x = np.asarray(inputs["x"], dtype=np.float32)
    a = np.asarray(inputs["a"], dtype=np.float32)
    w = np.asarray(inputs["kernel"], dtype=np.float32)
    assert x.shape == (B, N, F) and a.shape == (B, N, N) and w.shape == (F, D)

    nc = get_nc()
    in_maps = make_in_maps(x, a, w)
    res = run_bass_kernel_spmd(nc, in_maps, core_ids=list(range(B)))
    return np.stack(
        [np.asarray(res.results[b]["out"]).astype(np.float32) for b in range(B)],
        axis=0,
    )
